# revision 7
# baseline (speedup 1.0000x reference)
"""GAT-style edge-softmax self-attention, dense-mask formulation, 8-core SPMD.

Math per batch b (one NeuronCore per batch):
  Q/K/V = X @ Wq/k/v ; per head h: S = Q_h K_h^T / 8
  ex = C * exp(S)          (C[i,j] = multiplicity of edge (i<-j); softmax is
                            shift-invariant and |S|/8 <~ 6, so no row-max)
  out_i = (ex @ V)_i / max(sum_j ex_ij, 1e-9)

Implementation notes:
 - V is projected directly into a packed [N, 12*(64+1)] layout (per head: 64
   value columns + a ones column) using a host-prepacked weight wvp plus a
   rank-1 ones-row matmul, so the denominator falls out of the ex@V matmul.
 - S^T tiles [j,i] are computed with two heads packed into PE row-groups
   (K=64 quadrant tiling, concurrent matmuls).
 - exp runs on ACT over [128,1024] PSUM reads; the C-mask multiply runs
   in-place on DVE in bf16 (2x mode).
 - Output is staged as [N, 12*65] (65th columns hold junk); host strips them.
"""

import numpy as np
import ml_dtypes

import concourse.bass as bass
import concourse.bacc as bacc
import concourse.mybir as mybir
import concourse.tile as tile
from concourse.bass_utils import run_bass_kernel_spmd

B, N, H = 8, 1024, 768
NH, HD = 12, 64
P = 128
KC = H // P      # 6 contraction chunks
JC = N // P      # 8 node chunks
HW = HD + 1      # 65: head slot width (values + denominator)
VW = NH * HW     # 780
F32 = mybir.dt.float32
BF16 = mybir.dt.bfloat16

_CACHE = {}


def _build_nc():
    nc = bacc.Bacc("TRN2", target_bir_lowering=False, debug=True)

    xT_d = nc.dram_tensor("xT", [H, N], BF16, kind="ExternalInput")
    wq_d = nc.dram_tensor("wq", [H, H], BF16, kind="ExternalInput")
    wk_d = nc.dram_tensor("wk", [H, H], BF16, kind="ExternalInput")
    wvp_d = nc.dram_tensor("wvp", [H, VW], BF16, kind="ExternalInput")
    ones_d = nc.dram_tensor("ones", [1, P], BF16, kind="ExternalInput")
    erow_d = nc.dram_tensor("erow", [1, VW], BF16, kind="ExternalInput")
    mT_d = nc.dram_tensor("maskT", [N, N], BF16, kind="ExternalInput")
    out_d = nc.dram_tensor("out", [N, VW], F32, kind="ExternalOutput")

    with tile.TileContext(nc) as tc:
        with tc.tile_pool(name="res", bufs=1) as res, \
             tc.tile_pool(name="exm", bufs=20) as exmp, \
             tc.tile_pool(name="wrk", bufs=4) as wrk, \
             tc.tile_pool(name="sps", bufs=3, space="PSUM") as sps, \
             tc.tile_pool(name="ops", bufs=2, space="PSUM") as ops:

            # ---- resident loads ----
            xT = [res.tile([P, N], BF16, tag=f"xT{k}", name=f"xT{k}") for k in range(KC)]
            wq = [res.tile([P, H], BF16, tag=f"wq{k}", name=f"wq{k}") for k in range(KC)]
            wk = [res.tile([P, H], BF16, tag=f"wk{k}", name=f"wk{k}") for k in range(KC)]
            wvp = [res.tile([P, VW], BF16, tag=f"wvp{k}", name=f"wvp{k}") for k in range(KC)]
            mT = [res.tile([P, N], BF16, tag=f"mT{j}", name=f"mT{j}") for j in range(JC)]
            ones = res.tile([1, P], BF16, tag="ones", name="ones")
            erow = res.tile([1, VW], BF16, tag="erow", name="erow")
            nc.default_dma_engine.dma_start(out=ones[:], in_=ones_d[:, :])
            nc.default_dma_engine.dma_start(out=erow[:], in_=erow_d[:, :])
            for k in range(KC):
                nc.default_dma_engine.dma_start(out=xT[k][:], in_=xT_d[k * P:(k + 1) * P, :])
                nc.default_dma_engine.dma_start(out=wq[k][:], in_=wq_d[k * P:(k + 1) * P, :])
                nc.default_dma_engine.dma_start(out=wk[k][:], in_=wk_d[k * P:(k + 1) * P, :])
                nc.default_dma_engine.dma_start(out=wvp[k][:], in_=wvp_d[k * P:(k + 1) * P, :])
            for j in range(JC):
                nc.default_dma_engine.dma_start(out=mT[j][:], in_=mT_d[j * P:(j + 1) * P, :])

            # computed residents
            qT = [res.tile([P, N], BF16, tag=f"qT{k}", name=f"qT{k}") for k in range(KC)]
            kT = [res.tile([P, N], BF16, tag=f"kT{k}", name=f"kT{k}") for k in range(KC)]
            vp = [res.tile([P, VW], BF16, tag=f"vp{j}", name=f"vp{j}") for j in range(JC)]
            outt = res.tile([P, JC * VW], F32, tag="outt", name="outt")

            def proj_qk(w_sb, dst, mo):
                # k-outer so each stationary is loaded once for both halves
                ps = sps.tile([P, N], F32, tag="s", name="ps_qk")
                for k in range(KC):
                    for nn in range(2):
                        nc.tensor.matmul(
                            ps[:, nn * 512:(nn + 1) * 512],
                            w_sb[k][:, mo * P:(mo + 1) * P],
                            xT[k][:, nn * 512:(nn + 1) * 512],
                            start=(k == 0), stop=(k == KC - 1),
                        )
                nc.vector.tensor_copy(out=dst[mo][:], in_=ps[:])

            def proj_v(j):
                pv = sps.tile([P, VW], F32, tag="s", name="ps_v")
                for k in range(KC):
                    for c0, cw in ((0, 512), (512, VW - 512)):
                        nc.tensor.matmul(
                            pv[:, c0:c0 + cw],
                            xT[k][:, j * P:(j + 1) * P],
                            wvp[k][:, c0:c0 + cw],
                            start=(k == 0), stop=False,
                        )
                # += ones^T @ erow : writes 1.0 into the denominator cols
                for c0, cw in ((0, 512), (512, VW - 512)):
                    nc.tensor.matmul(
                        pv[:, c0:c0 + cw],
                        ones[0:1, :],
                        erow[0:1, c0:c0 + cw],
                        start=False, stop=True,
                    )
                nc.vector.tensor_copy(out=vp[j][:], in_=pv[:])

            # q0/k0 first so head-pair 0 can start, then all of V
            proj_qk(wq, qT, 0)
            proj_qk(wk, kT, 0)
            for j in range(JC):
                proj_v(j)

            # ---- main loop over head pairs ----
            for t in range(KC):
                exm = {}
                for j in range(JC):
                    psAB = [sps.tile([P, N], F32, tag="s", name=f"ps{t}_{j}_{ab}")
                            for ab in range(2)]
                    # interleave row-groups so the two heads' matmuls can
                    # run concurrently in the 64-row-tiled PE array
                    for i2 in range(2):
                        for ab in range(2):  # head 2t+ab in PE row-group ab
                            off = ab * HD
                            nc.tensor.matmul(
                                psAB[ab][:, i2 * 512:(i2 + 1) * 512],
                                kT[t][off:off + HD, j * P:(j + 1) * P],
                                qT[t][off:off + HD, i2 * 512:(i2 + 1) * 512],
                                start=True, stop=True,
                            )
                    for ab in range(2):
                        em = exmp.tile([P, N], BF16, tag="exm", name="em")
                        nc.scalar.activation(
                            em[:], psAB[ab][:],
                            mybir.ActivationFunctionType.Exp, scale=0.125)
                        eng = nc.gpsimd if j in (3, 7) else nc.vector
                        eng.tensor_tensor(
                            out=em[:], in0=em[:], in1=mT[j][:],
                            op=mybir.AluOpType.mult)
                        exm[(ab, j)] = em

                if t + 1 < KC:
                    proj_qk(wq, qT, t + 1)
                    proj_qk(wk, kT, t + 1)

                for ab in range(2):
                    h = 2 * t + ab
                    for i2 in range(2):
                        po = ops.tile([P, 4 * HW], F32, tag="o", name="po")
                        for j in range(JC):
                            for s in range(4):
                                nc.tensor.matmul(
                                    po[:, s * HW:(s + 1) * HW],
                                    exm[(ab, j)][:, (i2 * 4 + s) * P:(i2 * 4 + s + 1) * P],
                                    vp[j][:, h * HW:(h + 1) * HW],
                                    start=(j == 0 and s == 0),
                                    stop=(j == JC - 1 and s == 3),
                                )
                        # divide by the denominator column (clamped)
                        pv3 = po[:].rearrange("p (s c) -> p s c", c=HW)
                        den = wrk.tile([P, 4], F32, tag="den", name="den")
                        nc.vector.tensor_scalar_max(
                            den[:], pv3[:, :, HD:HD + 1].squeeze(2), 1e-9)
                        rec = wrk.tile([P, 4], F32, tag="rec", name="rec")
                        nc.vector.reciprocal(rec[:], den[:])
                        ov = outt[:].rearrange("p (ic c) -> p ic c", c=VW)[
                            :, i2 * 4:(i2 + 1) * 4, h * HW:(h + 1) * HW]
                        nc.vector.tensor_tensor(
                            out=ov, in0=pv3,
                            in1=rec[:].unsqueeze(2).broadcast_to([P, 4, HW]),
                            op=mybir.AluOpType.mult)

            for ic in range(JC):
                nc.default_dma_engine.dma_start(
                    out=out_d[ic * P:(ic + 1) * P, :],
                    in_=outt[:, ic * VW:(ic + 1) * VW])

    nc.compile()
    return nc


def _prep_in_maps(node_states, edge_indices, Wq, Wk, Wv):
    bf = ml_dtypes.bfloat16
    eb, ei, ej = (np.asarray(edge_indices[r]) for r in range(3))
    CT = np.zeros((B, N, N), dtype=np.float32)
    np.add.at(CT, (eb, ej, ei), 1.0)  # CT[b, j, i] = multiplicity of edge (i<-j)
    CTb = CT.astype(bf)

    wq = np.ascontiguousarray(Wq).astype(bf)
    wk = np.ascontiguousarray(Wk).astype(bf)
    wvp = np.zeros((H, VW), dtype=bf)
    wv = np.asarray(Wv)
    for h in range(NH):
        wvp[:, h * HW:h * HW + HD] = wv[:, h * HD:(h + 1) * HD].astype(bf)
    ones = np.ones((1, P), dtype=bf)
    erow = np.zeros((1, VW), dtype=bf)
    erow[0, HD::HW] = 1.0

    in_maps = []
    for b in range(B):
        in_maps.append({
            "xT": np.ascontiguousarray(np.asarray(node_states[b]).T).astype(bf),
            "wq": wq, "wk": wk, "wvp": wvp,
            "ones": ones, "erow": erow,
            "maskT": CTb[b],
        })
    return in_maps


def kernel(node_states, edge_indices, Wq, Wk, Wv):
    if "nc" not in _CACHE:
        _CACHE["nc"] = _build_nc()
    nc = _CACHE["nc"]

    in_maps = _prep_in_maps(node_states, edge_indices, Wq, Wk, Wv)
    res = run_bass_kernel_spmd(nc, in_maps, list(range(B)))
    out = np.stack([np.asarray(res.results[b]["out"]) for b in range(B)], axis=0)
    # strip the per-head denominator columns: [N, 12*65] -> [N, 12*64]
    out = out.reshape(B, N, NH, HW)[:, :, :, :HD].reshape(B, N, H)
    return np.ascontiguousarray(out).astype(np.float32)
